# revision 19
# baseline (speedup 1.0000x reference)
"""Gated Slot Attention (GSA) Trainium2 kernel, v3.

Sharding: B*H = 8 lanes -> 8 cores (core = b*4 + h). Each core computes its
lane's projections + chunked two-pass GLA recurrence, emitting the raw lane
output z = 2*o transposed [DV, T]. A second kernel applies silu + RMSNorm +
output projection with rows of (b,t) split across cores.

Chunked recurrence (C=256, all within one lane):
  Lam[i,m] = prod_{j<=i} g[j,m]  (= exp(-cumsum(softplus(-xf))/8))
  rlam = 1/Lam ; st_t = s_t/Lam_t = rlam_t - rlam_{t-1}
  ok   = Lam*(q @ Hk + mask(k^T q)^T St); qv = softmax_m(ok); qtt = qv*Lam
  o    = qtt @ Hv + mask(St qtt)^T v
  Hk' = Lend*(Hk + k^T St) ; Hv' = Lend*(Hv + St^T v)   (Lend pulled out)

The state sequence (Hk_c, Hv_c) depends only on projections+gates, never on
the softmax path, so per-chunk state snapshots are computed ahead and each
chunk's softmax->output path is an independent leaf chain. Emission is
software-pipelined: all f-projections+gates first (one ln-table residency),
then per-chunk stages P (qkv/transposes/states), Q (ok/exp, lag 1),
R (softmax-norm, lag 2), S (pass-2 output, lag 3) under the exp table:
exactly 2 activation-table loads for the whole kernel.

silu is synthesized as 2*silu(x) = (tanh(x/2)+1)*x; the 2x factors on q,k
cancel via the exp scale, the 2x on v rides through to kernel 2 where the
tanh scale absorbs it and RMSNorm cancels the rest.
"""
import sys
sys.path.insert(0, '/opt/trn_rl_repo')

import numpy as np
import ml_dtypes

import concourse.bass as bass
import concourse.bacc as bacc
import concourse.tile as tile
import concourse.mybir as mybir
import concourse.bass_utils as bass_utils

BF = mybir.dt.bfloat16
F32 = mybir.dt.float32
AF = mybir.ActivationFunctionType
OP = mybir.AluOpType

B, T, D = 2, 2048, 1024
H, DK, DV, M = 4, 256, 256, 256
C = 256            # chunk length
NCHUNK = T // C
NBATCH = NCHUNK // 2   # 2-chunk projection batches
GATE_NORM = 8.0
EPS = 1e-5

_cache = {}


def build_gsa():
    """Kernel 1: per-lane projections + chunked GLA. Output z [256, 2048] bf16
    (= 2*o, feature-major)."""
    nc = bacc.Bacc("TRN2", target_bir_lowering=False, debug=False, num_devices=8)
    hsT_d = nc.dram_tensor("hst", [D, T], BF, kind="ExternalInput").ap()
    w_d = nc.dram_tensor("wall", [D, 4 * 256], BF, kind="ExternalInput").ap()
    mask_d = nc.dram_tensor("mask", [C, C], BF, kind="ExternalInput").ap()
    ident_d = nc.dram_tensor("ident", [128, 128], BF, kind="ExternalInput").ap()
    z_d = nc.dram_tensor("z", [DV, T], BF, kind="ExternalOutput").ap()

    with tile.TileContext(nc) as tc:
        with (
            tc.tile_pool(name="persist", bufs=1) as pp,
            tc.tile_pool(name="hsp", bufs=4) as hsp,
            tc.tile_pool(name="gb", bufs=4) as gb,      # gate short-lived (batch)
            tc.tile_pool(name="gk", bufs=NBATCH) as gk,  # gate kept (batch)
            tc.tile_pool(name="qk", bufs=4) as qkp,     # qt/kt batch tiles
            tc.tile_pool(name="lv", bufs=5) as lv,      # per-chunk leaf tensors
            tc.tile_pool(name="sn", bufs=4) as snp,     # state snapshots
            tc.tile_pool(name="wk", bufs=3) as wk,      # short-lived
            tc.tile_pool(name="p512", bufs=2, space="PSUM") as p512,
            tc.tile_pool(name="p256", bufs=3, space="PSUM") as p256,
            tc.tile_pool(name="pT", bufs=1, space="PSUM") as pT,
            tc.tile_pool(name="pS", bufs=1, space="PSUM") as pS,
            tc.tile_pool(name="pB", bufs=1, space="PSUM") as pB,
        ):
            w = pp.tile([128, 8, 1024], BF, tag="w")
            msk = pp.tile([128, 2, C], BF, tag="msk")
            ident = pp.tile([128, 128], BF, tag="ident")
            ones_col = pp.tile([128, 1], BF, tag="onescol")
            ones_row = pp.tile([1, 128], BF, tag="onesrow")
            hkb0 = pp.tile([128, 2, 256], BF, tag="hkb0")
            hvb0 = pp.tile([128, 2, 256], BF, tag="hvb0")

            wv = w_d.rearrange("(a p) o -> p a o", p=128)
            hsv = hsT_d.rearrange("(a p) t -> p a t", p=128)
            # f weights first: the gate phase runs before everything else.
            # Split pieces let the first matmuls start as data lands.
            nc.sync.dma_start(out=w[:, 0:4, 768:1024], in_=wv[:, 0:4, 768:1024])
            nc.sync.dma_start(out=w[:, 4:8, 768:1024], in_=wv[:, 4:8, 768:1024])
            hs_t = {}
            for bt in range(NBATCH):
                hs_t[bt] = hsp.tile([128, 8, 512], BF, tag="hs", name="hs")
                for hh in range(2):
                    nc.sync.dma_start(
                        out=hs_t[bt][:, 4 * hh:4 * (hh + 1), :],
                        in_=hsv[:, 4 * hh:4 * (hh + 1), bt * 512:(bt + 1) * 512])
            nc.sync.dma_start(out=msk, in_=mask_d.rearrange("(a p) t -> p a t", p=128))
            nc.sync.dma_start(out=ident, in_=ident_d)
            nc.sync.dma_start(out=w[:, :, 0:768], in_=wv[:, :, 0:768])
            nc.vector.memset(ones_col, 1.0)
            nc.vector.memset(ones_row, 1.0)
            nc.gpsimd.memset(hkb0, 0.0)
            nc.gpsimd.memset(hvb0, 0.0)

            zv = z_d.rearrange("(a p) t -> p a t", p=128)

            Sb, lamb, stb, qtb, ktb = {}, {}, {}, {}, {}
            v_un, st_un, k_un, lbc, hkb, hvb, et, qtt = ({} for _ in range(8))

            # ---- phase F: f projections + gates for all batches.
            # Sub-passes keep same-table activations adjacent on Act:
            # exps (exp table), lns (ln table), exps again -> 3 loads total.
            # All 8 e^-xf tiles land in one big tile so the softplus ln is a
            # SINGLE activation instruction: exactly one natural_log table
            # residency regardless of scheduler interleaving (exp/tanh ops
            # share the other table).
            e1all = pp.tile([128, 8, 512], F32, tag="e1all")
            for bt in range(NBATCH):
                hs = hs_t[bt]
                for mt in range(2):
                    ps = p512.tile([128, 512], F32, tag="p512")
                    for dt in range(8):
                        nc.tensor.matmul(
                            ps, lhsT=w[:, dt, 768 + mt * 128:768 + (mt + 1) * 128],
                            rhs=hs[:, dt, :], start=(dt == 0), stop=(dt == 7))
                    nc.scalar.activation(e1all[:, bt * 2 + mt, :], ps, AF.Exp,
                                         scale=-1.0)
            # ln(e1 + 1) = softplus(-xf) = nsp, all batches at once
            nc.scalar.activation(e1all, e1all, AF.Ln, bias=1.0)
            for bt in range(NBATCH):
                e1 = e1all[:, bt * 2:bt * 2 + 2, :]
                rl = gb.tile([128, 2, 512], F32, tag="rl", name="rl")
                Sb[bt] = gk.tile([128, 2, 512], F32, tag="Sb", name="Sb")
                lamb[bt] = gk.tile([128, 2, 512], F32, tag="lamb", name="lamb")
                stb[bt] = gk.tile([128, 2, 512], BF, tag="stb", name="stb")
                for mt in range(2):
                    # e1 = nsp; per-chunk cumsum
                    nc.vector.tensor_tensor_scan(
                        Sb[bt][:, mt, 0:256], e1[:, mt, 0:256], e1[:, mt, 0:256],
                        0.0, OP.add, OP.bypass)
                    nc.vector.tensor_tensor_scan(
                        Sb[bt][:, mt, 256:512], e1[:, mt, 256:512],
                        e1[:, mt, 256:512], 0.0, OP.add, OP.bypass)
                    nc.scalar.activation(
                        rl[:, mt, :], Sb[bt][:, mt, :], AF.Exp,
                        scale=1.0 / GATE_NORM)
                    nc.vector.reciprocal(lamb[bt][:, mt, :], rl[:, mt, :])
                    # st_t = rlam_t - rlam_{t-1}; chunk-boundary cols use rlam=1
                    nc.gpsimd.tensor_tensor(
                        stb[bt][:, mt, 1:512], rl[:, mt, 1:512], rl[:, mt, 0:511],
                        op=OP.subtract)
                    for h2 in range(2):
                        nc.vector.tensor_scalar_sub(
                            stb[bt][:, mt, h2 * 256:h2 * 256 + 1],
                            rl[:, mt, h2 * 256:h2 * 256 + 1], 1.0)

            def chunk_views(c):
                bt, h2 = c // 2, c % 2
                off = h2 * 256
                stc = stb[bt][:, :, off:off + 256]
                lamc = lamb[bt][:, :, off:off + 256]
                qtc = qtb[bt][:, :, off:off + 256]
                ktc = ktb[bt][:, :, off:off + 256]
                return stc, lamc, qtc, ktc

            def stage_G(bt):
                """q/k/v projections + silu for one 2-chunk batch."""
                hs = hs_t[bt]
                qtb[bt] = qkp.tile([128, 2, 512], BF, tag="qtb", name="qtb")
                ktb[bt] = qkp.tile([128, 2, 512], BF, tag="ktb", name="ktb")
                for base, dst in ((0, qtb[bt]), (256, ktb[bt])):
                    for ot in range(2):
                        ps = p512.tile([128, 512], F32, tag="p512")
                        for dt in range(8):
                            nc.tensor.matmul(
                                ps,
                                lhsT=w[:, dt, base + ot * 128:base + (ot + 1) * 128],
                                rhs=hs[:, dt, :], start=(dt == 0), stop=(dt == 7))
                        th = wk.tile([128, 512], BF, tag="th")
                        nc.scalar.activation(th, ps, AF.Tanh, scale=0.5)
                        nc.vector.scalar_tensor_tensor(
                            out=dst[:, ot, :], in0=th, scalar=1.0, in1=ps,
                            op0=OP.add, op1=OP.mult)
                for h2 in range(2):
                    c = 2 * bt + h2
                    v_un[c] = lv.tile([128, 2, 256], BF, tag="vun", name="vun",
                                      bufs=8)
                    for tt in range(2):
                        ps = p256.tile([128, 256], F32, tag="p256")
                        for dt in range(8):
                            nc.tensor.matmul(
                                ps,
                                lhsT=hs[:, dt, h2 * 256 + tt * 128:h2 * 256 + (tt + 1) * 128],
                                rhs=w[:, dt, 512:768], start=(dt == 0), stop=(dt == 7))
                        th = wk.tile([128, 256], BF, tag="th2")
                        nc.scalar.activation(th, ps, AF.Tanh, scale=0.5)
                        nc.vector.scalar_tensor_tensor(
                            out=v_un[c][:, tt, :], in0=th, scalar=1.0, in1=ps,
                            op0=OP.add, op1=OP.mult)

            def stage_P(c):
                """Transposes, lend, state updates."""
                stc, lamc, qtc, ktc = chunk_views(c)
                # transposes: [tau, m | dk]: skun[:,lt,0:256]=st_un, 256:512=k_un
                skun = lv.tile([128, 2, 512], BF, tag="skun", name="skun")
                st_un[c] = skun[:, :, 0:256]
                k_un[c] = skun[:, :, 256:512]
                pst = pT.tile([128, 1024], BF, tag="pT")
                for lt in range(2):
                    for mt in range(2):
                        nc.tensor.transpose(
                            pst[:, lt * 512 + mt * 128:lt * 512 + (mt + 1) * 128],
                            stc[:, mt, lt * 128:(lt + 1) * 128], ident)
                    for k2 in range(2):
                        nc.tensor.transpose(
                            pst[:, lt * 512 + 256 + k2 * 128:lt * 512 + 256 + (k2 + 1) * 128],
                            ktc[:, k2, lt * 128:(lt + 1) * 128], ident)
                nc.scalar.activation(
                    skun.rearrange("p a b -> p (a b)"), pst, AF.Copy)

                # lend broadcast [p, m]
                lamcb = wk.tile([128, 2], BF, tag="lamcb")
                for mt in range(2):
                    nc.gpsimd.tensor_copy(lamcb[:, mt:mt + 1], lamc[:, mt, 255:256])
                plr = pT.tile([128, 256], BF, tag="pT")
                for mt in range(2):
                    nc.tensor.transpose(
                        plr[0:1, mt * 128:(mt + 1) * 128], lamcb[:, mt:mt + 1], ident)
                lrow = wk.tile([1, 256], BF, tag="lrow")
                nc.vector.tensor_copy(lrow, plr[0:1, :])
                pbc = pB.tile([128, 256], F32, tag="pB")
                nc.tensor.matmul(pbc, lhsT=ones_row, rhs=lrow, start=True, stop=True)
                lbc[c] = wk.tile([128, 256], BF, tag="lbc", name="lbc")
                nc.vector.tensor_copy(lbc[c], pbc)

                # state updates (bf16 chain, old state folded in via identity
                # matmul; Lend ~ e^-22 so bf16 rounding of the old state is
                # negligible): Hk_c = Lend*(Hk_{c-1} + k^T St)
                if c < NCHUNK - 1:
                    hkp = hkb[c - 1] if c > 0 else hkb0
                    hvp = hvb[c - 1] if c > 0 else hvb0
                    hkb[c] = snp.tile([128, 2, 256], BF, tag="hkb", name="hkb")
                    hvb[c] = snp.tile([128, 2, 256], BF, tag="hvb", name="hvb")
                    for dt2 in range(2):
                        ps = p256.tile([128, 256], F32, tag="p256")
                        for lt in range(2):
                            nc.tensor.matmul(
                                ps, lhsT=k_un[c][:, lt, dt2 * 128:(dt2 + 1) * 128],
                                rhs=st_un[c][:, lt, :], start=(lt == 0), stop=False)
                        nc.tensor.matmul(ps, lhsT=ident, rhs=hkp[:, dt2, :],
                                         start=False, stop=True)
                        nc.vector.tensor_tensor(hkb[c][:, dt2, :], ps, lbc[c],
                                                op=OP.mult)
                    for mt in range(2):
                        ps = p256.tile([128, 256], F32, tag="p256")
                        for lt in range(2):
                            nc.tensor.matmul(
                                ps, lhsT=st_un[c][:, lt, mt * 128:(mt + 1) * 128],
                                rhs=v_un[c][:, lt, :], start=(lt == 0), stop=False)
                        nc.tensor.matmul(ps, lhsT=ident, rhs=hvp[:, mt, :],
                                         start=False, stop=True)
                        nc.vector.tensor_scalar_mul(hvb[c][:, mt, :], ps,
                                                    lamc[:, mt, 255:256])

            def stage_Q(c):
                """Gram + ok + exp for chunk c (lag 1)."""
                stc, lamc, qtc, ktc = chunk_views(c)
                ptm = wk.tile([128, 2, C], BF, tag="ptm")
                for lt in range(2):
                    ps = p256.tile([128, C], F32, tag="p256")
                    for k2 in range(2):
                        nc.tensor.matmul(
                            ps, lhsT=ktc[:, k2, lt * 128:(lt + 1) * 128],
                            rhs=qtc[:, k2, :], start=(k2 == 0), stop=(k2 == 1))
                    nc.vector.tensor_tensor(ptm[:, lt, :], ps, msk[:, lt, :],
                                            op=OP.mult)
                hkp = hkb[c - 1] if c > 0 else hkb0
                et[c] = lv.tile([128, 2, C], BF, tag="et", name="et")
                for mt in range(2):
                    ps = p256.tile([128, C], F32, tag="p256")
                    for lt in range(2):
                        nc.tensor.matmul(
                            ps, lhsT=st_un[c][:, lt, mt * 128:(mt + 1) * 128],
                            rhs=ptm[:, lt, :], start=(lt == 0), stop=False)
                    for k2 in range(2):
                        nc.tensor.matmul(
                            ps, lhsT=hkp[:, k2, mt * 128:(mt + 1) * 128],
                            rhs=qtc[:, k2, :], start=False, stop=(k2 == 1))
                    # q,k each carry 2x from the tanh-silu -> exp scale 0.25
                    tmp = wk.tile([128, C], F32, tag="tmp")
                    nc.vector.tensor_tensor(tmp, lamc[:, mt, :], ps, op=OP.mult)
                    nc.scalar.activation(et[c][:, mt, :], tmp, AF.Exp, scale=0.25)

            def stage_R(c):
                """Softmax normalization for chunk c (lag 2)."""
                stc, lamc, qtc, ktc = chunk_views(c)
                cs = pS.tile([1, C], F32, tag="pS")
                for mt in range(2):
                    nc.tensor.matmul(cs, lhsT=ones_col, rhs=et[c][:, mt, :],
                                     start=(mt == 0), stop=(mt == 1))
                rrow = wk.tile([1, C], BF, tag="rrow")
                with nc.allow_low_precision(reason="softmax denom bcast in bf16"):
                    nc.vector.reciprocal(rrow, cs)
                bcr = pB.tile([128, C], F32, tag="pB")
                nc.tensor.matmul(bcr, lhsT=ones_row, rhs=rrow, start=True, stop=True)
                qtt[c] = lv.tile([128, 2, C], BF, tag="qtt", name="qtt")
                tmp2 = wk.tile([128, 2, C], BF, tag="tmp2")
                for mt in range(2):
                    nc.gpsimd.tensor_tensor(tmp2[:, mt, :], lamc[:, mt, :],
                                            et[c][:, mt, :], op=OP.mult)
                    nc.vector.tensor_tensor(qtt[c][:, mt, :], tmp2[:, mt, :], bcr,
                                            op=OP.mult)

            def stage_S(c):
                """Pass-2 output for chunk c (lag 3)."""
                stc, lamc, qtc, ktc = chunk_views(c)
                p2m = wk.tile([128, 2, C], BF, tag="p2m")
                for lt in range(2):
                    ps = p256.tile([128, C], F32, tag="p256")
                    for mt in range(2):
                        nc.tensor.matmul(
                            ps, lhsT=stc[:, mt, lt * 128:(lt + 1) * 128],
                            rhs=qtt[c][:, mt, :], start=(mt == 0), stop=(mt == 1))
                    nc.vector.tensor_tensor(p2m[:, lt, :], ps, msk[:, lt, :],
                                            op=OP.mult)
                hvp = hvb[c - 1] if c > 0 else hvb0
                zt = wk.tile([128, 2, C], BF, tag="zt")
                for vt in range(2):
                    ps = p256.tile([128, C], F32, tag="p256")
                    for lt in range(2):
                        nc.tensor.matmul(
                            ps, lhsT=v_un[c][:, lt, 128 * vt:128 * (vt + 1)],
                            rhs=p2m[:, lt, :], start=(lt == 0), stop=False)
                    for mt in range(2):
                        nc.tensor.matmul(
                            ps, lhsT=hvp[:, mt, vt * 128:(vt + 1) * 128],
                            rhs=qtt[c][:, mt, :], start=False, stop=(mt == 1))
                    nc.scalar.activation(zt[:, vt, :], ps, AF.Copy)
                nc.sync.dma_start(out=zv[:, :, c * C:(c + 1) * C], in_=zt)

            for bt in range(NBATCH):
                stage_G(bt)
            for it in range(NCHUNK + 3):
                if it < NCHUNK:
                    stage_P(it)
                if 1 <= it <= NCHUNK:
                    stage_Q(it - 1)
                if 2 <= it <= NCHUNK + 1:
                    stage_R(it - 2)
                if 3 <= it <= NCHUNK + 2:
                    stage_S(it - 3)
    nc.compile()
    return nc


def build_final():
    """Kernel 2: u = 4*silu(z/2); y = (u * rsqrt-bcast) @ wot.
    z [1024, 512] bf16 feature-major; wot host-folded (g_w, 1/4).
    Out yT [1024, 512] bf16."""
    nc = bacc.Bacc("TRN2", target_bir_lowering=False, debug=False, num_devices=8)
    z_d = nc.dram_tensor("zin", [D, 512], BF, kind="ExternalInput").ap()
    wo_d = nc.dram_tensor("wot", [D, D], BF, kind="ExternalInput").ap()
    y_d = nc.dram_tensor("y", [D, 512], BF, kind="ExternalOutput").ap()

    with tile.TileContext(nc) as tc:
        with (
            tc.tile_pool(name="sb", bufs=1) as sb,
            tc.tile_pool(name="yp", bufs=3) as yp,
            tc.tile_pool(name="ps", bufs=4, space="PSUM") as psp,
            tc.tile_pool(name="pss", bufs=1, space="PSUM") as pssp,
            tc.tile_pool(name="psb", bufs=1, space="PSUM") as psbp,
        ):
            z = sb.tile([128, 8, 512], BF, tag="z")
            wo = sb.tile([128, 8, 1024], BF, tag="wo")
            u = sb.tile([128, 8, 512], BF, tag="u")
            squ = sb.tile([128, 8, 512], BF, tag="squ")
            ones_col = sb.tile([128, 1], BF, tag="onescol")
            ones_row = sb.tile([1, 128], BF, tag="onesrow")
            nc.vector.memset(ones_col, 1.0)
            nc.vector.memset(ones_row, 1.0)
            zvw = z_d.rearrange("(a p) t -> p a t", p=128)
            wov = wo_d.rearrange("(a p) o -> p a o", p=128)
            # z and wo interleaved so y-matmuls stream ct-outer
            for ct in range(8):
                nc.sync.dma_start(out=z[:, ct, :], in_=zvw[:, ct, :])
                nc.sync.dma_start(out=wo[:, ct, :], in_=wov[:, ct, :])

            # silu: z = 2*o ; u = (tanh(o/2)+1)*z = 4*silu(o)
            # group-0 y matmuls run ct-outer, overlapped with silu + sumsq
            sqs = pssp.tile([1, 512], F32, tag="pss")
            yps = [psp.tile([128, 512], F32, tag="ps", name="yps")
                   for _ in range(4)]
            for ct in range(8):
                th = yp.tile([128, 512], BF, tag="th")
                nc.scalar.activation(th, z[:, ct, :], AF.Tanh, scale=0.25)
                nc.vector.scalar_tensor_tensor(
                    out=u[:, ct, :], in0=th, scalar=1.0, in1=z[:, ct, :],
                    op0=OP.add, op1=OP.mult)
                nc.gpsimd.tensor_tensor(squ[:, ct, :], u[:, ct, :], u[:, ct, :],
                                        op=OP.mult)
                nc.tensor.matmul(sqs, lhsT=ones_col, rhs=squ[:, ct, :],
                                 start=(ct == 0), stop=(ct == 7))
                for oi in range(4):
                    nc.tensor.matmul(
                        yps[oi], lhsT=wo[:, ct, oi * 128:(oi + 1) * 128],
                        rhs=u[:, ct, :], start=(ct == 0), stop=(ct == 7))
            # u = 4*silu -> mean(silu^2) = sqs/(16*1024); the 1/4 is in wot
            sq = sb.tile([1, 512], F32, tag="sq")
            eps_t = sb.tile([1, 1], F32, tag="epst")
            nc.vector.memset(eps_t, EPS)
            nc.scalar.activation(sq, sqs, AF.Sqrt, scale=1.0 / (16.0 * D), bias=eps_t)
            rr = sb.tile([1, 512], BF, tag="rr")
            with nc.allow_low_precision(reason="rms bcast in bf16"):
                nc.vector.reciprocal(rr, sq)
            bcr = psbp.tile([128, 512], F32, tag="psb")
            nc.tensor.matmul(bcr, lhsT=ones_row, rhs=rr, start=True, stop=True)
            rbb = sb.tile([128, 512], F32, tag="rbb")
            nc.vector.tensor_copy(rbb, bcr)

            yv = y_d.rearrange("(a p) t -> p a t", p=128)
            for oi in range(4):
                ysb = yp.tile([128, 512], BF, tag="ysb")
                nc.vector.tensor_tensor(ysb, yps[oi], rbb, op=OP.mult)
                nc.sync.dma_start(out=yv[:, oi, :], in_=ysb)
            # group 1: everything resident -> ot-outer so scales/stores stream
            for oi in range(4):
                yp2 = psp.tile([128, 512], F32, tag="ps", name="yps")
                for ct in range(8):
                    nc.tensor.matmul(
                        yp2, lhsT=wo[:, ct, 512 + oi * 128:512 + (oi + 1) * 128],
                        rhs=u[:, ct, :], start=(ct == 0), stop=(ct == 7))
                ysb = yp.tile([128, 512], BF, tag="ysb")
                nc.vector.tensor_tensor(ysb, yp2, rbb, op=OP.mult)
                nc.sync.dma_start(out=yv[:, 4 + oi, :], in_=ysb)
    nc.compile()
    return nc


def _get(name):
    if name not in _cache:
        _cache[name] = build_gsa() if name == "gsa" else build_final()
    return _cache[name]


def kernel(hidden_states, Wq, Wk, Wv, Wf, g_w, Wo, _trace=False):
    bf = ml_dtypes.bfloat16
    hidden_states = np.asarray(hidden_states, np.float32)
    Wq, Wk, Wv, Wf = (np.asarray(x, np.float32) for x in (Wq, Wk, Wv, Wf))
    g_w, Wo = np.asarray(g_w, np.float32), np.asarray(Wo, np.float32)

    mask = np.triu(np.ones((C, C), np.float32)).astype(bf)  # keep lambda <= tau
    ident = np.eye(128).astype(bf)
    in1 = []
    for core in range(8):
        b, h = core // 4, core % 4
        sl = slice(h * 256, (h + 1) * 256)
        wall = np.concatenate(
            [Wq[sl].T, Wk[sl].T, Wv[sl].T, Wf[sl].T], axis=1)   # [1024, 1024]
        in1.append({
            "hst": np.ascontiguousarray(hidden_states[b].T).astype(bf),
            "wall": np.ascontiguousarray(wall).astype(bf),
            "mask": mask,
            "ident": ident,
        })
    nc1 = _get("gsa")
    r1 = bass_utils.run_bass_kernel_spmd(nc1, in1, core_ids=list(range(8)),
                                         trace=_trace)
    zs = [r1.results[c]["z"] for c in range(8)]        # each [256, 2048] bf16

    # wot folds g_w and the 1/4 that de-scales u = 4*silu(o)
    wot = np.ascontiguousarray((Wo * (0.25 * g_w)[None, :]).T).astype(bf)
    in2 = []
    for core in range(8):
        b, q = core // 4, core % 4
        zb = np.concatenate([zs[b * 4 + hh] for hh in range(4)], axis=0)
        in2.append({
            "zin": np.ascontiguousarray(zb[:, q * 512:(q + 1) * 512]),
            "wot": wot,
        })
    nc2 = _get("final")
    r2 = bass_utils.run_bass_kernel_spmd(nc2, in2, core_ids=list(range(8)),
                                         trace=_trace)
    out = np.empty((B, T, D), np.float32)
    for core in range(8):
        b, q = core // 4, core % 4
        out[b, q * 512:(q + 1) * 512, :] = np.asarray(
            r2.results[core]["y"], np.float32).T
    if _trace:
        kernel.last_traces = (r1, r2)
    return out


# revision 21
# speedup vs baseline: 1.1035x; 1.1035x over previous
"""Gated Slot Attention (GSA) Trainium2 kernel, v3.

Sharding: B*H = 8 lanes -> 8 cores (core = b*4 + h). Each core computes its
lane's projections + chunked two-pass GLA recurrence, emitting the raw lane
output z = 2*o transposed [DV, T]. A second kernel applies silu + RMSNorm +
output projection with rows of (b,t) split across cores.

Chunked recurrence (C=256, all within one lane):
  Lam[i,m] = prod_{j<=i} g[j,m]  (= exp(-cumsum(softplus(-xf))/8))
  rlam = 1/Lam ; st_t = s_t/Lam_t = rlam_t - rlam_{t-1}
  ok   = Lam*(q @ Hk + mask(k^T q)^T St); qv = softmax_m(ok); qtt = qv*Lam
  o    = qtt @ Hv + mask(St qtt)^T v
  Hk' = Lend*(Hk + k^T St) ; Hv' = Lend*(Hv + St^T v)   (Lend pulled out)

The state sequence (Hk_c, Hv_c) depends only on projections+gates, never on
the softmax path, so per-chunk state snapshots are computed ahead and each
chunk's softmax->output path is an independent leaf chain. Emission is
software-pipelined: all f-projections+gates first (one ln-table residency),
then per-chunk stages P (qkv/transposes/states), Q (ok/exp, lag 1),
R (softmax-norm, lag 2), S (pass-2 output, lag 3) under the exp table:
exactly 2 activation-table loads for the whole kernel.

silu is synthesized as 2*silu(x) = (tanh(x/2)+1)*x; the 2x factors on q,k
cancel via the exp scale, the 2x on v rides through to kernel 2 where the
tanh scale absorbs it and RMSNorm cancels the rest.
"""
import sys
sys.path.insert(0, '/opt/trn_rl_repo')

import numpy as np
import ml_dtypes

import concourse.bass as bass
import concourse.bacc as bacc
import concourse.tile as tile
import concourse.mybir as mybir
import concourse.bass_utils as bass_utils

BF = mybir.dt.bfloat16
F32 = mybir.dt.float32
AF = mybir.ActivationFunctionType
OP = mybir.AluOpType

B, T, D = 2, 2048, 1024
H, DK, DV, M = 4, 256, 256, 256
C = 256            # chunk length
NCHUNK = T // C
NBATCH = NCHUNK // 2   # 2-chunk projection batches
GATE_NORM = 8.0
EPS = 1e-5

_cache = {}


def build_gsa():
    """Kernel 1: per-lane projections + chunked GLA. Output z [256, 2048] bf16
    (= 2*o, feature-major)."""
    nc = bacc.Bacc("TRN2", target_bir_lowering=False, debug=False, num_devices=8)
    hsT_d = nc.dram_tensor("hst", [D, T], BF, kind="ExternalInput").ap()
    w_d = nc.dram_tensor("wall", [D, 4 * 256], BF, kind="ExternalInput").ap()
    mask_d = nc.dram_tensor("mask", [C, C], BF, kind="ExternalInput").ap()
    ident_d = nc.dram_tensor("ident", [128, 128], BF, kind="ExternalInput").ap()
    z_d = nc.dram_tensor("z", [DV, T], BF, kind="ExternalOutput").ap()

    with tile.TileContext(nc) as tc:
        with (
            tc.tile_pool(name="persist", bufs=1) as pp,
            tc.tile_pool(name="hsp", bufs=4) as hsp,
            tc.tile_pool(name="gb", bufs=2) as gb,      # gate short-lived (batch)
            tc.tile_pool(name="gk", bufs=NBATCH) as gk,  # gate kept (batch)
            tc.tile_pool(name="qk", bufs=4) as qkp,     # qt/kt batch tiles
            tc.tile_pool(name="lv", bufs=8) as lv,      # per-chunk leaf tensors
            tc.tile_pool(name="sn", bufs=6) as snp,     # state snapshots
            tc.tile_pool(name="wk", bufs=3) as wk,      # short-lived
            tc.tile_pool(name="p512", bufs=2, space="PSUM") as p512,
            tc.tile_pool(name="p256", bufs=3, space="PSUM") as p256,
            tc.tile_pool(name="pT", bufs=1, space="PSUM") as pT,
            tc.tile_pool(name="pS", bufs=1, space="PSUM") as pS,
            tc.tile_pool(name="pB", bufs=1, space="PSUM") as pB,
        ):
            w = pp.tile([128, 8, 1024], BF, tag="w")
            msk = pp.tile([128, 2, C], BF, tag="msk")
            ident = pp.tile([128, 128], BF, tag="ident")
            ones_col = pp.tile([128, 1], BF, tag="onescol")
            ones_row = pp.tile([1, 128], BF, tag="onesrow")
            hkb0 = pp.tile([128, 2, 256], BF, tag="hkb0")
            hvb0 = pp.tile([128, 2, 256], BF, tag="hvb0")

            wv = w_d.rearrange("(a p) o -> p a o", p=128)
            hsv = hsT_d.rearrange("(a p) t -> p a t", p=128)
            # f weights first: the gate phase runs before everything else.
            # Split pieces let the first matmuls start as data lands.
            nc.sync.dma_start(out=w[:, 0:4, 768:1024], in_=wv[:, 0:4, 768:1024])
            nc.sync.dma_start(out=w[:, 4:8, 768:1024], in_=wv[:, 4:8, 768:1024])
            hs_t = {}
            for bt in range(NBATCH):
                hs_t[bt] = hsp.tile([128, 8, 512], BF, tag="hs", name="hs")
                for hh in range(2):
                    nc.sync.dma_start(
                        out=hs_t[bt][:, 4 * hh:4 * (hh + 1), :],
                        in_=hsv[:, 4 * hh:4 * (hh + 1), bt * 512:(bt + 1) * 512])
            nc.sync.dma_start(out=msk, in_=mask_d.rearrange("(a p) t -> p a t", p=128))
            nc.sync.dma_start(out=ident, in_=ident_d)
            nc.sync.dma_start(out=w[:, :, 0:768], in_=wv[:, :, 0:768])
            nc.vector.memset(ones_col, 1.0)
            nc.vector.memset(ones_row, 1.0)
            nc.gpsimd.memset(hkb0, 0.0)
            nc.gpsimd.memset(hvb0, 0.0)

            zv = z_d.rearrange("(a p) t -> p a t", p=128)

            Sb, lamb, stb, qtb, ktb = {}, {}, {}, {}, {}
            v_un, st_un, k_un, lbc, hkb, hvb, et, qtt = ({} for _ in range(8))

            # ---- phase F: f projections + gates for all batches.
            # Sub-passes keep same-table activations adjacent on Act:
            # exps (exp table), lns (ln table), exps again -> 3 loads total.
            # All 8 e^-xf tiles land in one big tile so the softplus ln is a
            # SINGLE activation instruction: exactly one natural_log table
            # residency regardless of scheduler interleaving (exp/tanh ops
            # share the other table).
            e1all = pp.tile([128, 8, 512], F32, tag="e1all")
            for bt in range(NBATCH):
                hs = hs_t[bt]
                for mt in range(2):
                    ps = p512.tile([128, 512], F32, tag="p512")
                    for dt in range(8):
                        nc.tensor.matmul(
                            ps, lhsT=w[:, dt, 768 + mt * 128:768 + (mt + 1) * 128],
                            rhs=hs[:, dt, :], start=(dt == 0), stop=(dt == 7))
                    nc.scalar.activation(e1all[:, bt * 2 + mt, :], ps, AF.Exp,
                                         scale=-1.0)
            def stage_F3(bt):
                """Post-ln gate math for one batch: cumsum, rlam, lam, st."""
                e1 = e1all[:, bt * 2:bt * 2 + 2, :]
                rl = gb.tile([128, 2, 512], F32, tag="rl", name="rl")
                Sb[bt] = gb.tile([128, 2, 512], F32, tag="Sb", name="Sb")
                lamb[bt] = gk.tile([128, 2, 512], F32, tag="lamb", name="lamb")
                stb[bt] = gk.tile([128, 2, 512], BF, tag="stb", name="stb")
                for mt in range(2):
                    # e1 = nsp; per-chunk cumsum
                    nc.vector.tensor_tensor_scan(
                        Sb[bt][:, mt, 0:256], e1[:, mt, 0:256], e1[:, mt, 0:256],
                        0.0, OP.add, OP.bypass)
                    nc.vector.tensor_tensor_scan(
                        Sb[bt][:, mt, 256:512], e1[:, mt, 256:512],
                        e1[:, mt, 256:512], 0.0, OP.add, OP.bypass)
                    nc.scalar.activation(
                        rl[:, mt, :], Sb[bt][:, mt, :], AF.Exp,
                        scale=1.0 / GATE_NORM)
                    nc.vector.reciprocal(lamb[bt][:, mt, :], rl[:, mt, :])
                    # st_t = rlam_t - rlam_{t-1}; chunk-boundary cols use rlam=1
                    nc.gpsimd.tensor_tensor(
                        stb[bt][:, mt, 1:512], rl[:, mt, 1:512], rl[:, mt, 0:511],
                        op=OP.subtract)
                    for h2 in range(2):
                        nc.vector.tensor_scalar_sub(
                            stb[bt][:, mt, h2 * 256:h2 * 256 + 1],
                            rl[:, mt, h2 * 256:h2 * 256 + 1], 1.0)

            def chunk_views(c):
                bt, h2 = c // 2, c % 2
                off = h2 * 256
                stc = stb[bt][:, :, off:off + 256]
                lamc = lamb[bt][:, :, off:off + 256]
                qtc = qtb[bt][:, :, off:off + 256]
                ktc = ktb[bt][:, :, off:off + 256]
                return stc, lamc, qtc, ktc

            def stage_G(bt):
                """q/k/v projections + silu for one 2-chunk batch."""
                hs = hs_t[bt]
                qtb[bt] = qkp.tile([128, 2, 512], BF, tag="qtb", name="qtb")
                ktb[bt] = qkp.tile([128, 2, 512], BF, tag="ktb", name="ktb")
                for base, dst in ((0, qtb[bt]), (256, ktb[bt])):
                    for ot in range(2):
                        ps = p512.tile([128, 512], F32, tag="p512")
                        for dt in range(8):
                            nc.tensor.matmul(
                                ps,
                                lhsT=w[:, dt, base + ot * 128:base + (ot + 1) * 128],
                                rhs=hs[:, dt, :], start=(dt == 0), stop=(dt == 7))
                        th = wk.tile([128, 512], BF, tag="th")
                        nc.scalar.activation(th, ps, AF.Tanh, scale=0.5)
                        nc.vector.scalar_tensor_tensor(
                            out=dst[:, ot, :], in0=th, scalar=1.0, in1=ps,
                            op0=OP.add, op1=OP.mult)
                for h2 in range(2):
                    c = 2 * bt + h2
                    v_un[c] = lv.tile([128, 2, 256], BF, tag="vun", name="vun",
                                      bufs=8)
                    for tt in range(2):
                        ps = p256.tile([128, 256], F32, tag="p256")
                        for dt in range(8):
                            nc.tensor.matmul(
                                ps,
                                lhsT=hs[:, dt, h2 * 256 + tt * 128:h2 * 256 + (tt + 1) * 128],
                                rhs=w[:, dt, 512:768], start=(dt == 0), stop=(dt == 7))
                        th = wk.tile([128, 256], BF, tag="th2")
                        nc.scalar.activation(th, ps, AF.Tanh, scale=0.5)
                        nc.vector.scalar_tensor_tensor(
                            out=v_un[c][:, tt, :], in0=th, scalar=1.0, in1=ps,
                            op0=OP.add, op1=OP.mult)

            def stage_P(c):
                """Transposes, lend, state updates."""
                stc, lamc, qtc, ktc = chunk_views(c)
                # transposes: [tau, m | dk]: skun[:,lt,0:256]=st_un, 256:512=k_un
                skun = lv.tile([128, 2, 512], BF, tag="skun", name="skun")
                st_un[c] = skun[:, :, 0:256]
                k_un[c] = skun[:, :, 256:512]
                pst = pT.tile([128, 1024], BF, tag="pT")
                for lt in range(2):
                    for mt in range(2):
                        nc.tensor.transpose(
                            pst[:, lt * 512 + mt * 128:lt * 512 + (mt + 1) * 128],
                            stc[:, mt, lt * 128:(lt + 1) * 128], ident)
                    for k2 in range(2):
                        nc.tensor.transpose(
                            pst[:, lt * 512 + 256 + k2 * 128:lt * 512 + 256 + (k2 + 1) * 128],
                            ktc[:, k2, lt * 128:(lt + 1) * 128], ident)
                nc.scalar.activation(
                    skun.rearrange("p a b -> p (a b)"), pst, AF.Copy)

                # lend broadcast [p, m]
                lamcb = wk.tile([128, 2], BF, tag="lamcb")
                for mt in range(2):
                    nc.gpsimd.tensor_copy(lamcb[:, mt:mt + 1], lamc[:, mt, 255:256])
                plr = pT.tile([128, 256], BF, tag="pT")
                for mt in range(2):
                    nc.tensor.transpose(
                        plr[0:1, mt * 128:(mt + 1) * 128], lamcb[:, mt:mt + 1], ident)
                lrow = wk.tile([1, 256], BF, tag="lrow")
                nc.vector.tensor_copy(lrow, plr[0:1, :])
                pbc = pB.tile([128, 256], F32, tag="pB")
                nc.tensor.matmul(pbc, lhsT=ones_row, rhs=lrow, start=True, stop=True)
                lbc[c] = wk.tile([128, 256], BF, tag="lbc", name="lbc")
                nc.vector.tensor_copy(lbc[c], pbc)

                # state updates (bf16 chain, old state folded in via identity
                # matmul; Lend ~ e^-22 so bf16 rounding of the old state is
                # negligible): Hk_c = Lend*(Hk_{c-1} + k^T St)
                if c < NCHUNK - 1:
                    hkp = hkb[c - 1] if c > 0 else hkb0
                    hvp = hvb[c - 1] if c > 0 else hvb0
                    hkb[c] = snp.tile([128, 2, 256], BF, tag="hkb", name="hkb")
                    hvb[c] = snp.tile([128, 2, 256], BF, tag="hvb", name="hvb")
                    for dt2 in range(2):
                        ps = p256.tile([128, 256], F32, tag="p256")
                        for lt in range(2):
                            nc.tensor.matmul(
                                ps, lhsT=k_un[c][:, lt, dt2 * 128:(dt2 + 1) * 128],
                                rhs=st_un[c][:, lt, :], start=(lt == 0), stop=False)
                        nc.tensor.matmul(ps, lhsT=ident, rhs=hkp[:, dt2, :],
                                         start=False, stop=True)
                        nc.vector.tensor_tensor(hkb[c][:, dt2, :], ps, lbc[c],
                                                op=OP.mult)
                    for mt in range(2):
                        ps = p256.tile([128, 256], F32, tag="p256")
                        for lt in range(2):
                            nc.tensor.matmul(
                                ps, lhsT=st_un[c][:, lt, mt * 128:(mt + 1) * 128],
                                rhs=v_un[c][:, lt, :], start=(lt == 0), stop=False)
                        nc.tensor.matmul(ps, lhsT=ident, rhs=hvp[:, mt, :],
                                         start=False, stop=True)
                        nc.vector.tensor_scalar_mul(hvb[c][:, mt, :], ps,
                                                    lamc[:, mt, 255:256])

            def stage_Q(c):
                """Gram + ok + exp for chunk c (lag 1)."""
                stc, lamc, qtc, ktc = chunk_views(c)
                ptm = wk.tile([128, 2, C], BF, tag="ptm")
                for lt in range(2):
                    ps = p256.tile([128, C], F32, tag="p256")
                    for k2 in range(2):
                        nc.tensor.matmul(
                            ps, lhsT=ktc[:, k2, lt * 128:(lt + 1) * 128],
                            rhs=qtc[:, k2, :], start=(k2 == 0), stop=(k2 == 1))
                    nc.vector.tensor_tensor(ptm[:, lt, :], ps, msk[:, lt, :],
                                            op=OP.mult)
                hkp = hkb[c - 1] if c > 0 else hkb0
                et[c] = lv.tile([128, 2, C], BF, tag="et", name="et")
                for mt in range(2):
                    ps = p256.tile([128, C], F32, tag="p256")
                    for lt in range(2):
                        nc.tensor.matmul(
                            ps, lhsT=st_un[c][:, lt, mt * 128:(mt + 1) * 128],
                            rhs=ptm[:, lt, :], start=(lt == 0), stop=False)
                    for k2 in range(2):
                        nc.tensor.matmul(
                            ps, lhsT=hkp[:, k2, mt * 128:(mt + 1) * 128],
                            rhs=qtc[:, k2, :], start=False, stop=(k2 == 1))
                    # q,k each carry 2x from the tanh-silu -> exp scale 0.25
                    tmp = wk.tile([128, C], F32, tag="tmp")
                    nc.vector.tensor_tensor(tmp, lamc[:, mt, :], ps, op=OP.mult)
                    nc.scalar.activation(et[c][:, mt, :], tmp, AF.Exp, scale=0.25)

            def stage_R(c):
                """Softmax normalization for chunk c (lag 2)."""
                stc, lamc, qtc, ktc = chunk_views(c)
                cs = pS.tile([1, C], F32, tag="pS")
                for mt in range(2):
                    nc.tensor.matmul(cs, lhsT=ones_col, rhs=et[c][:, mt, :],
                                     start=(mt == 0), stop=(mt == 1))
                rrow = wk.tile([1, C], BF, tag="rrow")
                with nc.allow_low_precision(reason="softmax denom bcast in bf16"):
                    nc.vector.reciprocal(rrow, cs)
                bcr = pB.tile([128, C], F32, tag="pB")
                nc.tensor.matmul(bcr, lhsT=ones_row, rhs=rrow, start=True, stop=True)
                qtt[c] = lv.tile([128, 2, C], BF, tag="qtt", name="qtt")
                tmp2 = wk.tile([128, 2, C], BF, tag="tmp2")
                for mt in range(2):
                    nc.gpsimd.tensor_tensor(tmp2[:, mt, :], lamc[:, mt, :],
                                            et[c][:, mt, :], op=OP.mult)
                    nc.vector.tensor_tensor(qtt[c][:, mt, :], tmp2[:, mt, :], bcr,
                                            op=OP.mult)

            def stage_S(c):
                """Pass-2 output for chunk c (lag 3)."""
                stc, lamc, qtc, ktc = chunk_views(c)
                p2m = wk.tile([128, 2, C], BF, tag="p2m")
                for lt in range(2):
                    ps = p256.tile([128, C], F32, tag="p256")
                    for mt in range(2):
                        nc.tensor.matmul(
                            ps, lhsT=stc[:, mt, lt * 128:(lt + 1) * 128],
                            rhs=qtt[c][:, mt, :], start=(mt == 0), stop=(mt == 1))
                    nc.vector.tensor_tensor(p2m[:, lt, :], ps, msk[:, lt, :],
                                            op=OP.mult)
                hvp = hvb[c - 1] if c > 0 else hvb0
                zt = wk.tile([128, 2, C], BF, tag="zt")
                for vt in range(2):
                    ps = p256.tile([128, C], F32, tag="p256")
                    for lt in range(2):
                        nc.tensor.matmul(
                            ps, lhsT=v_un[c][:, lt, 128 * vt:128 * (vt + 1)],
                            rhs=p2m[:, lt, :], start=(lt == 0), stop=False)
                    for mt in range(2):
                        nc.tensor.matmul(
                            ps, lhsT=hvp[:, mt, vt * 128:(vt + 1) * 128],
                            rhs=qtt[c][:, mt, :], start=False, stop=(mt == 1))
                    nc.scalar.activation(zt[:, vt, :], ps, AF.Copy)
                nc.sync.dma_start(out=zv[:, :, c * C:(c + 1) * C], in_=zt)

            stage_G(0)
            # ln(e1 + 1) = softplus(-xf) = nsp, all batches in one instruction
            # (single natural_log residency; everything else shares exp/tanh)
            nc.scalar.activation(e1all, e1all, AF.Ln, bias=1.0)
            stage_F3(0)
            for bt in range(1, NBATCH):
                stage_G(bt)
                stage_F3(bt)
            for it in range(NCHUNK + 6):
                if it < NCHUNK:
                    stage_P(it)
                if 2 <= it < NCHUNK + 2:
                    stage_Q(it - 2)
                if 4 <= it < NCHUNK + 4:
                    stage_R(it - 4)
                if 6 <= it < NCHUNK + 6:
                    stage_S(it - 6)
    nc.compile()
    return nc


def build_final():
    """Kernel 2: u = 4*silu(z/2); y = (u * rsqrt-bcast) @ wot.
    z [1024, 512] bf16 feature-major; wot host-folded (g_w, 1/4).
    Out yT [1024, 512] bf16."""
    nc = bacc.Bacc("TRN2", target_bir_lowering=False, debug=False, num_devices=8)
    z_d = nc.dram_tensor("zin", [D, 512], BF, kind="ExternalInput").ap()
    wo_d = nc.dram_tensor("wot", [D, D], BF, kind="ExternalInput").ap()
    y_d = nc.dram_tensor("y", [D, 512], BF, kind="ExternalOutput").ap()

    with tile.TileContext(nc) as tc:
        with (
            tc.tile_pool(name="sb", bufs=1) as sb,
            tc.tile_pool(name="yp", bufs=3) as yp,
            tc.tile_pool(name="ps", bufs=4, space="PSUM") as psp,
            tc.tile_pool(name="pss", bufs=1, space="PSUM") as pssp,
            tc.tile_pool(name="psb", bufs=1, space="PSUM") as psbp,
        ):
            z = sb.tile([128, 8, 512], BF, tag="z")
            wo = sb.tile([128, 8, 1024], BF, tag="wo")
            u = sb.tile([128, 8, 512], BF, tag="u")
            squ = sb.tile([128, 8, 512], BF, tag="squ")
            ones_col = sb.tile([128, 1], BF, tag="onescol")
            ones_row = sb.tile([1, 128], BF, tag="onesrow")
            nc.vector.memset(ones_col, 1.0)
            nc.vector.memset(ones_row, 1.0)
            zvw = z_d.rearrange("(a p) t -> p a t", p=128)
            wov = wo_d.rearrange("(a p) o -> p a o", p=128)
            # z and wo interleaved so y-matmuls stream ct-outer
            for ct in range(8):
                nc.sync.dma_start(out=z[:, ct, :], in_=zvw[:, ct, :])
                nc.sync.dma_start(out=wo[:, ct, :], in_=wov[:, ct, :])

            # silu: z = 2*o ; u = (tanh(o/2)+1)*z = 4*silu(o)
            # group-0 y matmuls run ct-outer, overlapped with silu + sumsq
            sqs = pssp.tile([1, 512], F32, tag="pss")
            yps = [psp.tile([128, 512], F32, tag="ps", name="yps")
                   for _ in range(4)]
            for ct in range(8):
                th = yp.tile([128, 512], BF, tag="th")
                nc.scalar.activation(th, z[:, ct, :], AF.Tanh, scale=0.25)
                nc.vector.scalar_tensor_tensor(
                    out=u[:, ct, :], in0=th, scalar=1.0, in1=z[:, ct, :],
                    op0=OP.add, op1=OP.mult)
                nc.gpsimd.tensor_tensor(squ[:, ct, :], u[:, ct, :], u[:, ct, :],
                                        op=OP.mult)
                nc.tensor.matmul(sqs, lhsT=ones_col, rhs=squ[:, ct, :],
                                 start=(ct == 0), stop=(ct == 7))
                for oi in range(4):
                    nc.tensor.matmul(
                        yps[oi], lhsT=wo[:, ct, oi * 128:(oi + 1) * 128],
                        rhs=u[:, ct, :], start=(ct == 0), stop=(ct == 7))
            # u = 4*silu -> mean(silu^2) = sqs/(16*1024); the 1/4 is in wot
            sq = sb.tile([1, 512], F32, tag="sq")
            eps_t = sb.tile([1, 1], F32, tag="epst")
            nc.vector.memset(eps_t, EPS)
            nc.scalar.activation(sq, sqs, AF.Sqrt, scale=1.0 / (16.0 * D), bias=eps_t)
            rr = sb.tile([1, 512], BF, tag="rr")
            with nc.allow_low_precision(reason="rms bcast in bf16"):
                nc.vector.reciprocal(rr, sq)
            bcr = psbp.tile([128, 512], F32, tag="psb")
            nc.tensor.matmul(bcr, lhsT=ones_row, rhs=rr, start=True, stop=True)
            rbb = sb.tile([128, 512], F32, tag="rbb")
            nc.vector.tensor_copy(rbb, bcr)

            yv = y_d.rearrange("(a p) t -> p a t", p=128)
            for oi in range(4):
                ysb = yp.tile([128, 512], BF, tag="ysb")
                nc.vector.tensor_tensor(ysb, yps[oi], rbb, op=OP.mult)
                nc.sync.dma_start(out=yv[:, oi, :], in_=ysb)
            # group 1: everything resident -> ot-outer so scales/stores stream
            for oi in range(4):
                yp2 = psp.tile([128, 512], F32, tag="ps", name="yps")
                for ct in range(8):
                    nc.tensor.matmul(
                        yp2, lhsT=wo[:, ct, 512 + oi * 128:512 + (oi + 1) * 128],
                        rhs=u[:, ct, :], start=(ct == 0), stop=(ct == 7))
                ysb = yp.tile([128, 512], BF, tag="ysb")
                nc.vector.tensor_tensor(ysb, yp2, rbb, op=OP.mult)
                nc.sync.dma_start(out=yv[:, 4 + oi, :], in_=ysb)
    nc.compile()
    return nc


def _get(name):
    if name not in _cache:
        _cache[name] = build_gsa() if name == "gsa" else build_final()
    return _cache[name]


def kernel(hidden_states, Wq, Wk, Wv, Wf, g_w, Wo, _trace=False):
    bf = ml_dtypes.bfloat16
    hidden_states = np.asarray(hidden_states, np.float32)
    Wq, Wk, Wv, Wf = (np.asarray(x, np.float32) for x in (Wq, Wk, Wv, Wf))
    g_w, Wo = np.asarray(g_w, np.float32), np.asarray(Wo, np.float32)

    mask = np.triu(np.ones((C, C), np.float32)).astype(bf)  # keep lambda <= tau
    ident = np.eye(128).astype(bf)
    in1 = []
    for core in range(8):
        b, h = core // 4, core % 4
        sl = slice(h * 256, (h + 1) * 256)
        wall = np.concatenate(
            [Wq[sl].T, Wk[sl].T, Wv[sl].T, Wf[sl].T], axis=1)   # [1024, 1024]
        in1.append({
            "hst": np.ascontiguousarray(hidden_states[b].T).astype(bf),
            "wall": np.ascontiguousarray(wall).astype(bf),
            "mask": mask,
            "ident": ident,
        })
    nc1 = _get("gsa")
    r1 = bass_utils.run_bass_kernel_spmd(nc1, in1, core_ids=list(range(8)),
                                         trace=_trace)
    zs = [r1.results[c]["z"] for c in range(8)]        # each [256, 2048] bf16

    # wot folds g_w and the 1/4 that de-scales u = 4*silu(o)
    wot = np.ascontiguousarray((Wo * (0.25 * g_w)[None, :]).T).astype(bf)
    in2 = []
    for core in range(8):
        b, q = core // 4, core % 4
        zb = np.concatenate([zs[b * 4 + hh] for hh in range(4)], axis=0)
        in2.append({
            "zin": np.ascontiguousarray(zb[:, q * 512:(q + 1) * 512]),
            "wot": wot,
        })
    nc2 = _get("final")
    r2 = bass_utils.run_bass_kernel_spmd(nc2, in2, core_ids=list(range(8)),
                                         trace=_trace)
    out = np.empty((B, T, D), np.float32)
    for core in range(8):
        b, q = core // 4, core % 4
        out[b, q * 512:(q + 1) * 512, :] = np.asarray(
            r2.results[core]["y"], np.float32).T
    if _trace:
        kernel.last_traces = (r1, r2)
    return out
